# revision 39
# baseline (speedup 1.0000x reference)
"""Trainium2 Bass kernel for Transformer-XL-style relative attention.

nn module: x[1,2048,1024], 16 heads, depth 64; causal attention with
Music-Transformer skewed relative-position bias (q @ E^T + pad/reshape skew),
then output projection.

Sharding: tensor-parallel over heads, 2 heads per core on 8 cores, no
on-chip collectives. Each core computes its heads' attention and its partial
c_proj contribution; the host sums the 8 partials (the TP output reduction,
done during the un-shard gather) and adds c_proj_b.

Per-core dataflow, all in "transposed" [j, q] orientation so the attention
probabilities feed the ctx matmul directly with no transposes:
  qkvT = Wqkv^T @ x^T           bf16 matmuls, f32 PSUM accum (+ qkv bias via
                                per-partition tensor_scalar on the evacuation)
  sT[j,i] = k_j . q_i           lhsT = kT tile, rhs = qT chunk; the two heads
                                run as K=64 matmuls on disjoint PE row groups
                                (concurrent) into the two banks of one wide
                                PSUM tile
  R[i,m] = q_i . E_m            written to DRAM (fp8e4m3 - E is Glorot-small
                                so the relative bias tolerates fp8) with row
                                stride S+1: the pad-trick skew. Reading the
                                same buffer with row stride S yields
                                bias[i,j] = R[i, S-1-i+j] as plain contiguous
                                "strips" [q, j] - no transposing DMA needed
  sT += strip.T                 per 128-col piece via identity-rhs matmuls:
                                the matmul itself performs the transpose
  pT = exp(sT/8)                one wide ACT pass per head pair (bounded
                                logits - no row-max needed)
  causal: only tiles with 128*jt <= i are computed; the diagonal 128-col
          block is masked with a triangular 0/1 tile after exp
  ctxT & rowsum = [v | 1]^T @ pT   fused ones-column = softmax denominator
  1/rowsum on the raw [1, q] rowsum row (DVE approx), then PE-broadcast of
  the reciprocal across 64 partitions; ctxTn = ctxT * bcast. Head 1 is
  moved to partitions 64-127 by a SBUF->SBUF DMA (engines cannot cross
  partitions; DMA can).
  out_partial = ctxTn^T @ Wproj per 128-row q tile, bf16 partials to HBM,
  one DMA per 128-row tile so the tail drains early.

The whole thing is a single fused pipeline: x loads stream on 4 DMA queues
while the qkv projection consumes chunks as they land, and attention chunk
qc starts as soon as qkv chunks 0..qc and its bias strips are ready. This
keeps the PE busy continuously (the HAM clock gate halves the PE clock
after ~3.4us of idle, so gaps are poison).
"""

import math

import numpy as np
import ml_dtypes

import concourse.bacc as bacc
import concourse.bass as bass
import concourse.mybir as mybir
from concourse import tile
from concourse.bass_utils import run_bass_kernel_spmd

BF16 = mybir.dt.bfloat16
FP8 = mybir.dt.float8e4
F32 = mybir.dt.float32
AF = mybir.ActivationFunctionType

S = 2048          # sequence length
HID = 1024        # hidden
D = 64            # head depth
T = 128           # tile edge (q rows / j cols)
CH = 512          # q-chunk width (one PSUM bank of f32)
NQC = S // CH     # 4 q-chunks
NJT = S // T      # 16 j tiles
KC = HID // T     # 8 hidden chunks
PAD = T - 1
EW = S + T        # eT padded width (2176); cols >= S are zeros
NCORES = 8
BSZ = 1048704     # skew scratch elems per (head, q-chunk)


def _m0(t):
    return S - T * (t + 1)


def _mhi(t):
    # last q tile is trimmed to m <= S so adjacent-row writes in the skew
    # buffer stay collision-free on every causally-read position
    return S + 1 if t == NJT - 1 else S + T - 1


def build(debug=False):
    nc = bacc.Bacc()
    xT = nc.declare_dram_parameter("xT", [HID, S], BF16, False)
    wqkv = nc.declare_dram_parameter("wqkv", [HID, 3 * T], BF16, False)
    wqkv_b = nc.declare_dram_parameter("wqkv_b", [T, 3], F32, False)
    eT = nc.declare_dram_parameter("eT", [T, EW], BF16, False)
    wproj = nc.declare_dram_parameter("wproj", [T, HID], BF16, False)
    tri = nc.declare_dram_parameter("tri", [T, T], BF16, False)
    iden = nc.declare_dram_parameter("iden", [T, T], BF16, False)
    iden8 = nc.declare_dram_parameter("iden8", [T, T], FP8, False)
    out = nc.declare_dram_parameter("out", [S, HID], BF16, True)
    bsk = [[nc.dram_tensor(f"bsk{h}_{qc}", [BSZ], FP8) for qc in range(NQC)]
           for h in range(2)]
    if debug:
        d_qkv = nc.declare_dram_parameter("d_qkv", [T, 3 * S], BF16, True)
        d_v = nc.declare_dram_parameter("d_v", [T, NJT * 130], BF16, True)
        d_ctxn = nc.declare_dram_parameter("d_ctxn", [T, S], BF16, True)
        d_cxs = nc.declare_dram_parameter("d_cxs", [T, 2 * CH], F32, True)
        d_rb = nc.declare_dram_parameter("d_rb", [T, 2 * CH], F32, True)
        d_pt = nc.declare_dram_parameter("d_pt", [T, 2 * CH], BF16, True)

    with tile.TileContext(nc) as tc:
        with tc.tile_pool(name="const", bufs=1) as cp:
            xc_sb = [cp.tile([T, KC, CH], BF16, tag=f"x{nt}",
                             name=f"xc_sb{nt}") for nt in range(NQC)]
            wq_sb = cp.tile([T, KC, 3 * T], BF16, tag="wq")
            wqb_sb = cp.tile([T, 3], F32, tag="wqb")
            eT_sb = cp.tile([T, EW], BF16, tag="eT")
            wp_sb = cp.tile([T, HID], BF16, tag="wp")
            tri_sb = cp.tile([T, T], BF16, tag="tri")
            id_sb = cp.tile([T, T], BF16, tag="iden")
            id8_sb = cp.tile([T, T], FP8, tag="iden8")
            qkvT_sb = cp.tile([T, 3, S], BF16, tag="qkvT")
            # per j-tile: [v_h0(0:64) | ones(64) | v_h1(65:129) | ones(129)]
            v_sb = cp.tile([T, NJT, 130], BF16, tag="v")
            ctxn_sb = cp.tile([T, S], BF16, tag="ctxn")
            warm_sb = cp.tile([T, 2], F32, tag="warm")
            ones_sb = cp.tile([T, 64], BF16, tag="ones")

            # ---- input loads: one 3D-AP DMA per x chunk (a per-kc split
            # costs ~0.6us of issue time per DMA), spread over the 3 DMA
            # queues so x streams in while the qkv matmuls chase the chunks
            def x_load(eng, nt, k0=0, k1=KC):
                eng.dma_start(
                    out=xc_sb[nt][:, k0:k1, :],
                    in_=bass.AP(xT, T * k0 * S + nt * CH,
                                [[S, T], [T * S, k1 - k0], [1, CH]]))

            def wq_load(eng, k0, k1):
                eng.dma_start(
                    out=wq_sb[:, k0:k1, :],
                    in_=bass.AP(wqkv, T * k0 * 3 * T,
                                [[3 * T, T], [T * 3 * T, k1 - k0],
                                 [1, 3 * T]]))

            # the first qkv matmuls gate on wq[kc0..] + x0[kc0..]: split the
            # early tensors across two queues so no single ring serializes
            # the pipeline start
            nc.scalar.dma_start(out=wqb_sb[:], in_=wqkv_b[:])
            wq_load(nc.scalar, 0, 2)
            x_load(nc.scalar, 0, 0, 4)
            nc.scalar.dma_start(out=eT_sb[:], in_=eT[:])
            x_load(nc.scalar, 1, 0, 4)
            x_load(nc.scalar, 2)
            wq_load(nc.sync, 2, KC)
            nc.sync.dma_start(out=id_sb[:], in_=iden[:])
            nc.sync.dma_start(out=id8_sb[:], in_=iden8[:])
            x_load(nc.sync, 0, 4, KC)
            x_load(nc.sync, 1, 4, KC)
            nc.sync.dma_start(out=tri_sb[:], in_=tri[:])
            # gpsimd (slow software-DMA issue path): only late-needed loads
            x_load(nc.gpsimd, 3)
            nc.gpsimd.dma_start(out=wp_sb[:], in_=wproj[:])

            # preload the Exp table while the big DMAs run
            nc.vector.memset(warm_sb[:, 0:1], 0.0)
            nc.scalar.activation(warm_sb[:, 1:2], warm_sb[:, 0:1], AF.Exp)

            # ones columns for the fused rowsum
            nc.vector.memset(v_sb[:, :, 64:65], 1.0)
            nc.vector.memset(v_sb[:, :, 129:130], 1.0)
            nc.vector.memset(ones_sb[:], 1.0)

            with (
                tc.tile_pool(name="psS", bufs=2, space="PSUM") as psS,
                tc.tile_pool(name="psC", bufs=1, space="PSUM") as psC,
                tc.tile_pool(name="psA", bufs=2, space="PSUM") as psA,
                tc.tile_pool(name="sbR", bufs=4) as sbR,
                tc.tile_pool(name="sbB", bufs=2) as sbB,
                tc.tile_pool(name="sbP", bufs=8) as sbP,
                tc.tile_pool(name="sbN", bufs=2) as sbN,
                tc.tile_pool(name="sbO", bufs=4) as sbO,
            ):
                def qkv_chunk(nt):
                    for m in range(3):
                        acc = psA.tile([T, CH], F32, tag="aux",
                                       name=f"qkv{nt}_{m}")
                        for kc in range(KC):
                            nc.tensor.matmul(
                                acc[:],
                                wq_sb[:, kc, m * T:(m + 1) * T],
                                xc_sb[nt][:, kc, :],
                                start=(kc == 0), stop=(kc == KC - 1))
                        nc.vector.tensor_scalar_add(
                            qkvT_sb[:, m, nt * CH:(nt + 1) * CH], acc[:],
                            wqb_sb[:, m:m + 1])

                def vtr_chunk(nt):
                    # transpose this chunk's 4 v tiles into [j, d] layout
                    pv = psA.tile([T, CH], F32, tag="aux", name=f"vtr{nt}")
                    for jl in range(4):
                        jt = 4 * nt + jl
                        nc.tensor.matmul(
                            pv[:, jl * T:(jl + 1) * T],
                            qkvT_sb[:, 2, jt * T:(jt + 1) * T], id_sb[:],
                            start=True, stop=True, skip_group_check=True)
                    for jl in range(4):
                        jt = 4 * nt + jl
                        nc.vector.tensor_copy(v_sb[:, jt, 0:64],
                                              pv[:, jl * T:jl * T + 64])
                        nc.scalar.activation(v_sb[:, jt, 65:129],
                                             pv[:, jl * T + 64:jl * T + 128],
                                             AF.Copy)

                strip_tiles = {}

                def emit_R(qc, rs=(0, 1, 2, 3)):
                    if qc not in strip_tiles:
                        jwu = T * (4 * qc + 4)
                        strip_tiles[qc] = [
                            sbB.tile([T, 4, jwu], FP8, tag=f"strip{h}",
                                     name=f"stb{qc}_{h}") for h in range(2)]
                    jwu = T * (4 * qc + 4)
                    wq_ = [nc.sync, nc.gpsimd]        # skew DMA queue per head
                    for r in rs:
                        t = 4 * qc + r
                        m0, mhi = _m0(t), _mhi(t)
                        W = mhi - m0
                        jwt = S - m0          # causal band width
                        eRp = sbR.tile([T, 2, EW], FP8, tag="rawR",
                                       name=f"eRp{qc}_{r}")
                        for ck in range(math.ceil(jwt / CH)):
                            c0 = m0 + ck * CH
                            csz = min(CH, S - c0)
                            rp = [psA.tile([T, CH], F32, tag="aux",
                                           name=f"R{qc}_{r}_{ck}_{h}")
                                  for h in range(2)]
                            for h in range(2):
                                hp = slice(64 * h, 64 * h + 64)
                                nc.tensor.matmul(
                                    rp[h][:, 0:csz],
                                    qkvT_sb[hp, 0, t * T:(t + 1) * T],
                                    eT_sb[hp, c0:c0 + csz],
                                    start=True, stop=True,
                                    skip_group_check=True)
                            for h in range(2):
                                nc.vector.tensor_copy(
                                    eRp[:, h, ck * CH:ck * CH + csz],
                                    rp[h][:, 0:csz])
                        for h in range(2):
                            # non-causal tail -> exp ~ 0: the causal mask
                            # rides the skew for free (tri only for t=15,
                            # whose band has no room for a mask tail)
                            [nc.gpsimd, nc.vector][h].memset(
                                eRp[:, h, jwt:W], -200.0)
                            off_w = (PAD + CH * qc + T * r * (S + 1)
                                     + m0 - (S - 1))
                            wq_[h].dma_start(
                                out=bass.AP(bsk[h][qc], off_w,
                                            [[S + 1, T], [1, W]]),
                                in_=eRp[:, h, 0:W])
                            # bias strip r: same queue as its skew write, so
                            # it lands as soon as the write completes (and a
                            # chunk ahead of its consumers)
                            wq_[h].dma_start(
                                out=strip_tiles[qc][h][:, r, 0:jwu],
                                in_=bass.AP(bsk[h][qc], PAD + r * T * S,
                                            [[S, T], [1, jwu]]))

                def attn_chunk(qc, carry=(), next_qc=None, last=False):
                    # `carry` holds the previous chunk's deferred normalize +
                    # c_proj emission stages: the PE queue is strictly
                    # in-order, so emitting them at the boundary would stall
                    # the PE on the (long-latency) normalize chain while this
                    # chunk's ready scores sit behind it in the queue.
                    # Instead they interleave into the first few iterations.
                    strips = strip_tiles.pop(qc)
                    if carry:
                        carry[0]()
                    cx0 = psC.tile([T, CH], F32, tag="ctx0", name=f"cx0_{qc}")
                    cx1 = psC.tile([T, CH], F32, tag="ctx1", name=f"cx1_{qc}")
                    ctx_ps = [cx0, cx1]
                    jt_max = 4 * qc + 3

                    def emit_ctx(jt, il0, ext, pTp):
                        for h in range(2):
                            cx = ctx_ps[h]
                            nc.tensor.matmul(
                                cx[0:65, il0:il0 + ext],
                                v_sb[:, jt, 65 * h:65 * h + 65],
                                pTp[:, h, 0:ext],
                                start=(jt == 0), stop=(jt == jt_max),
                                skip_group_check=True)

                    pend = None
                    for jt in range(jt_max + 1):
                        if next_qc is not None and jt < 4:
                            # next chunk's R tiles, front-loaded so the skew
                            # DMA roundtrip completes well before that chunk
                            emit_R(next_qc, (jt,))
                        if 1 <= jt <= len(carry) - 1:
                            carry[jt]()
                        i0 = max(CH * qc, T * jt)
                        ext = CH * (qc + 1) - i0
                        il0 = i0 - CH * qc
                        diag = (i0 == T * jt)
                        r_lo = max(jt - 4 * qc, 0)
                        spp = psS.tile([T, 2 * CH], F32, tag="sT2",
                                       name=f"spp{qc}_{jt}")
                        for h in range(2):
                            hp = slice(64 * h, 64 * h + 64)
                            nc.tensor.matmul(
                                spp[:, h * CH:h * CH + ext],
                                qkvT_sb[hp, 1, jt * T:(jt + 1) * T],
                                qkvT_sb[hp, 0, i0:i0 + ext],
                                start=True, stop=False,
                                skip_group_check=True)
                        for h in range(2):
                            # bias add: strip[:, jt-tile].T via identity rhs,
                            # one 128-col piece per q tile covered by sp
                            for r in range(r_lo, 4):
                                co = h * CH + r * T - il0
                                nc.tensor.matmul(
                                    spp[:, co:co + T],
                                    strips[h][:, r, jt * T:(jt + 1) * T],
                                    id8_sb[:],
                                    start=False, stop=(r == 3),
                                    skip_group_check=True)
                        # ctx for the previous j tile: its exp completes
                        # while this tile's matmuls stream, so the PE never
                        # waits on the ACT engine
                        if pend is not None:
                            emit_ctx(*pend)
                        pTp = sbP.tile([T, 2, CH], BF16, tag="pT",
                                       name=f"pT{qc}_{jt}")
                        nc.scalar.activation(
                            pTp[:, :, 0:ext],
                            spp[:].rearrange("p (h c) -> p h c", h=2)
                            [:, :, 0:ext],
                            AF.Exp, scale=0.125)
                        if diag and jt == NJT - 1:
                            nc.vector.tensor_mul(pTp[:, 0, 0:T],
                                                 pTp[:, 0, 0:T], tri_sb[:])
                            nc.gpsimd.tensor_mul(pTp[:, 1, 0:T],
                                                 pTp[:, 1, 0:T], tri_sb[:])
                        if debug and qc == 0 and jt == 0:
                            nc.sync.dma_start(
                                out=bass.AP(d_pt, 0, [[2 * CH, T], [1, 2 * CH]]),
                                in_=pTp[:].rearrange("p a b -> p (a b)"))
                        pend = (jt, il0, ext, pTp)
                    emit_ctx(*pend)

                    # --- deferred normalize + merge heads + c_proj ---
                    def fin0():
                        # evacuate ctx PSUM (raw, incl. the rowsum row) right
                        # at the boundary so the next chunk's ctx
                        # accumulation gets its PSUM banks back immediately
                        for h in range(2):
                            nc.vector.tensor_copy(cxs[h][0:65, :],
                                                  ctx_ps[h][0:65, :])

                    def fin1(c0=0, c1=CH, shift_q=nc.sync):
                        # broadcast the RAW rowsum row across 64 partitions
                        # on PE (ones[1,64].T @ rowsum[1,512]), then the
                        # approx reciprocal runs 64-lane-parallel on PSUM
                        cw = c1 - c0
                        rsc = sbN.tile([T, 2, CH], BF16, tag="rsc",
                                       name=f"rsc{qc}_{c0}")
                        for h in range(2):
                            nc.vector.tensor_copy(rsc[64:65, h, c0:c1],
                                                  cxs[h][64:65, c0:c1])
                        bcp = [psA.tile([T, CH], F32, tag="aux",
                                        name=f"bcp{qc}_{h}_{c0}")
                               for h in range(2)]
                        bc = sbN.tile([T, 2, CH], F32, tag="bc",
                                      name=f"bc{qc}_{c0}")
                        for h in range(2):
                            nc.tensor.matmul(bcp[h][0:64, 0:cw],
                                             ones_sb[64:65, 0:64],
                                             rsc[64:65, h, c0:c1],
                                             start=True, stop=True)
                            nc.vector.reciprocal_approx_fast(
                                bc[0:64, h, c0:c1], bcp[h][0:64, 0:cw])
                        if debug and qc == 0 and c0 == 0:
                            for h in range(2):
                                nc.sync.dma_start(
                                    out=bass.AP(d_cxs, h * CH,
                                                [[2 * CH, T], [1, CH]]),
                                    in_=cxs[h][:])
                            nc.sync.dma_start(
                                out=bass.AP(d_rb, 0,
                                            [[2 * CH, T], [1, 2 * CH]]),
                                in_=bc[:].rearrange("p a b -> p (a b)"))
                        nc.vector.tensor_mul(
                            ctxn_sb[0:64, qc * CH + c0:qc * CH + c1],
                            cxs[0][0:64, c0:c1], bc[0:64, 0, c0:c1])
                        tmp1 = sbN.tile([T, CH], BF16, tag="tmp1",
                                        name=f"tmp1_{qc}_{c0}")
                        nc.vector.tensor_mul(tmp1[0:64, c0:c1],
                                             cxs[1][0:64, c0:c1],
                                             bc[0:64, 1, c0:c1])
                        # head 1 lives on partitions 64-127 of ctxn for the
                        # merged-head c_proj; engines cannot cross partitions
                        # but a SBUF->SBUF DMA can
                        shift_q.dma_start(
                            out=ctxn_sb[64:128, qc * CH + c0:qc * CH + c1],
                            in_=tmp1[0:64, c0:c1])

                    def cproj(rs_):
                        for r in rs_:
                            q0 = qc * CH + r * T
                            og = sbO.tile([T, HID], BF16, tag="og",
                                          name=f"og{qc}_{r}")
                            for oc in range(2):
                                pp = psA.tile([T, CH], F32, tag="aux",
                                              name=f"pp{qc}_{r}_{oc}")
                                nc.tensor.matmul(
                                    pp[:], ctxn_sb[:, q0:q0 + T],
                                    wp_sb[:, oc * CH:(oc + 1) * CH],
                                    start=True, stop=True)
                                if oc == 0:
                                    nc.vector.tensor_copy(
                                        og[:, oc * CH:(oc + 1) * CH], pp[:])
                                else:
                                    nc.scalar.activation(
                                        og[:, oc * CH:(oc + 1) * CH], pp[:],
                                        AF.Copy)
                            # last-processed chunk goes via the hardware DMA
                            # queues (sync/scalar) - the gpsimd software-DMA
                            # drain at NEFF teardown is slow
                            q_ = [nc.sync, nc.scalar] if qc == 0 \
                                else [nc.sync, nc.gpsimd]
                            q_[r % len(q_)].dma_start(
                                out=bass.AP(out, q0 * HID,
                                            [[HID, T], [1, HID]]),
                                in_=og[:])

                    cxs = [sbN.tile([T, CH], F32, tag=f"cxs{h}",
                                    name=f"cxs{qc}_{h}") for h in range(2)]
                    if last:
                        # final chunk: nothing left to interleave with, so
                        # pipeline the normalize by halves to unblock c_proj
                        # (and its out DMAs) as early as possible
                        return [fin0,
                                lambda: (fin1(0, CH // 2, nc.sync),
                                        cproj((0, 1))),
                                lambda: (fin1(CH // 2, CH, nc.scalar),
                                        cproj((2, 3)))]
                    return [fin0, fin1,
                            lambda: cproj((0, 1)), lambda: cproj((2, 3))]

                # ---- the fused pipeline ----
                # chunk order 2,3,0,1: any order is legal (chunks are
                # independent given qkv + their strips); the tiny chunk 0
                # sits mid-pipeline where its 4 j-tiles only need to absorb
                # prefetch work, and the final carry lands in chunk 1's
                # 8-iteration loop instead of dangling off the end
                qkv_chunk(0)
                qkv_chunk(1)
                qkv_chunk(2)
                emit_R(2)
                vtr_chunk(0)
                vtr_chunk(1)
                vtr_chunk(2)
                qkv_chunk(3)
                fin = attn_chunk(2, next_qc=3)
                vtr_chunk(3)
                fin = attn_chunk(3, fin, next_qc=0)
                fin = attn_chunk(0, fin, next_qc=1)
                fin = attn_chunk(1, fin, last=True)
                for f in fin:
                    f()

                if debug:
                    nc.sync.dma_start(
                        out=d_qkv[:],
                        in_=qkvT_sb[:].rearrange("p a b -> p (a b)"))
                    nc.sync.dma_start(
                        out=d_v[:], in_=v_sb[:].rearrange("p a b -> p (a b)"))
                    nc.sync.dma_start(out=d_ctxn[:], in_=ctxn_sb[:])

    nc.finalize()
    return nc


_NC_CACHE = {}


def _get_nc():
    if "nc" not in _NC_CACHE:
        _NC_CACHE["nc"] = build()
    return _NC_CACHE["nc"]


def _prep_core_inputs(x, c_attn_w, c_attn_b, c_proj_w, E):
    bf = ml_dtypes.bfloat16
    xT = np.ascontiguousarray(np.asarray(x)[0].T).astype(bf)     # [1024, 2048]
    c_attn_w = np.asarray(c_attn_w)
    c_attn_b = np.asarray(c_attn_b)
    c_proj_w = np.asarray(c_proj_w)
    E = np.asarray(E)
    # tri[j, q] = 1 if j <= q else 0 (upper triangular incl diagonal)
    tri = np.triu(np.ones((T, T), np.float32)).astype(bf)
    iden = np.eye(T, dtype=np.float32).astype(bf)
    iden8 = np.eye(T, dtype=np.float32).astype(ml_dtypes.float8_e4m3)
    maps = []
    for c in range(NCORES):
        qs = slice(T * c, T * (c + 1))
        wq = np.concatenate([
            c_attn_w[:, qs],
            c_attn_w[:, HID + T * c:HID + T * (c + 1)],
            c_attn_w[:, 2 * HID + T * c:2 * HID + T * (c + 1)],
        ], axis=1).astype(bf)                                    # [1024, 384]
        wqb = np.stack([
            c_attn_b[0, qs],
            c_attn_b[0, HID + T * c:HID + T * (c + 1)],
            c_attn_b[0, 2 * HID + T * c:2 * HID + T * (c + 1)],
        ], axis=1).astype(np.float32)                            # [128, 3]
        eTc = np.zeros((T, EW), np.float32)
        eTc[0:64, 0:S] = E[2 * c].T
        eTc[64:128, 0:S] = E[2 * c + 1].T
        wp = c_proj_w[T * c:T * (c + 1), :].astype(bf)           # [128, 1024]
        maps.append({
            "xT": xT, "wqkv": wq, "wqkv_b": wqb, "eT": eTc.astype(bf),
            "wproj": wp, "tri": tri, "iden": iden, "iden8": iden8,
        })
    return maps


def run_cores(inputs, trace=False, trace_kwargs=None):
    nc = _get_nc()
    maps = _prep_core_inputs(inputs["x"], inputs["c_attn_w"],
                             inputs["c_attn_b"], inputs["c_proj_w"],
                             inputs["E"])
    kw = {}
    if trace:
        kw["trace"] = True
        if trace_kwargs:
            kw.update(trace_kwargs)
    return run_bass_kernel_spmd(nc, maps, core_ids=list(range(NCORES)), **kw)


def kernel(**inputs):
    res = run_cores(inputs, trace=False)
    acc = np.zeros((S, HID), np.float32)
    for c in range(NCORES):
        acc += np.asarray(res.results[c]["out"]).astype(np.float32)
    acc += np.asarray(inputs["c_proj_b"]).astype(np.float32)
    return acc.reshape(1, S, HID)


# revision 41
# speedup vs baseline: 1.0880x; 1.0880x over previous
"""Trainium2 Bass kernel for Transformer-XL-style relative attention.

nn module: x[1,2048,1024], 16 heads, depth 64; causal attention with
Music-Transformer skewed relative-position bias (q @ E^T + pad/reshape skew),
then output projection.

Sharding: tensor-parallel over heads, 2 heads per core on 8 cores, no
on-chip collectives. Each core computes its heads' attention and its partial
c_proj contribution; the host sums the 8 partials (the TP output reduction,
done during the un-shard gather) and adds c_proj_b.

Per-core dataflow, all in "transposed" [j, q] orientation so the attention
probabilities feed the ctx matmul directly with no transposes:
  qkvT = Wqkv^T @ x^T           bf16 matmuls, f32 PSUM accum (+ qkv bias via
                                per-partition tensor_scalar on the evacuation)
  sT[j,i] = k_j . q_i           lhsT = kT tile, rhs = qT chunk; the two heads
                                run as K=64 matmuls on disjoint PE row groups
                                (concurrent) into the two banks of one wide
                                PSUM tile
  R[i,m] = q_i . E_m            written to DRAM (fp8e4m3 - E is Glorot-small
                                so the relative bias tolerates fp8) with row
                                stride S+1: the pad-trick skew. Reading the
                                same buffer with row stride S yields
                                bias[i,j] = R[i, S-1-i+j] as plain contiguous
                                "strips" [q, j] - no transposing DMA needed
  sT += strip.T                 per 128-col piece via identity-rhs matmuls:
                                the matmul itself performs the transpose
  pT = exp(sT/8)                one wide ACT pass per head pair (bounded
                                logits - no row-max needed)
  causal: only tiles with 128*jt <= i are computed; the diagonal 128-col
          block is masked with a triangular 0/1 tile after exp
  ctxT & rowsum = [v | 1]^T @ pT   fused ones-column = softmax denominator
  1/rowsum on the raw [1, q] rowsum row (DVE approx), then PE-broadcast of
  the reciprocal across 64 partitions; ctxTn = ctxT * bcast. Head 1 is
  moved to partitions 64-127 by a SBUF->SBUF DMA (engines cannot cross
  partitions; DMA can).
  out_partial = ctxTn^T @ Wproj per 128-row q tile, bf16 partials to HBM,
  one DMA per 128-row tile so the tail drains early.

The whole thing is a single fused pipeline: x loads stream on 4 DMA queues
while the qkv projection consumes chunks as they land, and attention chunk
qc starts as soon as qkv chunks 0..qc and its bias strips are ready. This
keeps the PE busy continuously (the HAM clock gate halves the PE clock
after ~3.4us of idle, so gaps are poison).
"""

import math

import numpy as np
import ml_dtypes

import concourse.bacc as bacc
import concourse.bass as bass
import concourse.mybir as mybir
from concourse import tile
from concourse.bass_utils import run_bass_kernel_spmd

BF16 = mybir.dt.bfloat16
FP8 = mybir.dt.float8e4
F32 = mybir.dt.float32
AF = mybir.ActivationFunctionType

S = 2048          # sequence length
HID = 1024        # hidden
D = 64            # head depth
T = 128           # tile edge (q rows / j cols)
CH = 512          # q-chunk width (one PSUM bank of f32)
NQC = S // CH     # 4 q-chunks
NJT = S // T      # 16 j tiles
KC = HID // T     # 8 hidden chunks
PAD = T - 1
EW = S + T        # eT padded width (2176); cols >= S are zeros
NCORES = 8
BSZ = 1048704     # skew scratch elems per (head, q-chunk)


def _m0(t):
    return S - T * (t + 1)


def _mhi(t):
    # last q tile is trimmed to m <= S so adjacent-row writes in the skew
    # buffer stay collision-free on every causally-read position
    return S + 1 if t == NJT - 1 else S + T - 1


def build(debug=False):
    nc = bacc.Bacc()
    xT = nc.declare_dram_parameter("xT", [HID, S], BF16, False)
    wqkv = nc.declare_dram_parameter("wqkv", [HID, 3 * T], BF16, False)
    wqkv_b = nc.declare_dram_parameter("wqkv_b", [T, 3], F32, False)
    eT = nc.declare_dram_parameter("eT", [T, EW], BF16, False)
    wproj = nc.declare_dram_parameter("wproj", [T, HID], BF16, False)
    tri = nc.declare_dram_parameter("tri", [T, T], BF16, False)
    iden = nc.declare_dram_parameter("iden", [T, T], BF16, False)
    iden8 = nc.declare_dram_parameter("iden8", [T, T], FP8, False)
    out = nc.declare_dram_parameter("out", [S, HID], BF16, True)
    bsk = [[nc.dram_tensor(f"bsk{h}_{qc}", [BSZ], FP8) for qc in range(NQC)]
           for h in range(2)]
    if debug:
        d_qkv = nc.declare_dram_parameter("d_qkv", [T, 3 * S], BF16, True)
        d_v = nc.declare_dram_parameter("d_v", [T, NJT * 130], BF16, True)
        d_ctxn = nc.declare_dram_parameter("d_ctxn", [T, S], BF16, True)
        d_cxs = nc.declare_dram_parameter("d_cxs", [T, 2 * CH], F32, True)
        d_rb = nc.declare_dram_parameter("d_rb", [T, 2 * CH], F32, True)
        d_pt = nc.declare_dram_parameter("d_pt", [T, 2 * CH], BF16, True)

    with tile.TileContext(nc) as tc:
        with tc.tile_pool(name="const", bufs=1) as cp:
            xc_sb = [cp.tile([T, KC, CH], BF16, tag=f"x{nt}",
                             name=f"xc_sb{nt}") for nt in range(NQC)]
            wq_sb = cp.tile([T, KC, 3 * T], BF16, tag="wq")
            wqb_sb = cp.tile([T, 3], F32, tag="wqb")
            eT_sb = cp.tile([T, EW], BF16, tag="eT")
            wp_sb = cp.tile([T, HID], BF16, tag="wp")
            tri_sb = cp.tile([T, T], BF16, tag="tri")
            id_sb = cp.tile([T, T], BF16, tag="iden")
            id8_sb = cp.tile([T, T], FP8, tag="iden8")
            qkvT_sb = cp.tile([T, 3, S], BF16, tag="qkvT")
            # per j-tile: [v_h0(0:64) | ones(64) | v_h1(65:129) | ones(129)]
            v_sb = cp.tile([T, NJT, 130], BF16, tag="v")
            ctxn_sb = cp.tile([T, S], BF16, tag="ctxn")
            warm_sb = cp.tile([T, 2], F32, tag="warm")
            ones_sb = cp.tile([T, 64], BF16, tag="ones")

            # ---- input loads: one 3D-AP DMA per x chunk (a per-kc split
            # costs ~0.6us of issue time per DMA), spread over the 3 DMA
            # queues so x streams in while the qkv matmuls chase the chunks
            def x_load(eng, nt, k0=0, k1=KC):
                eng.dma_start(
                    out=xc_sb[nt][:, k0:k1, :],
                    in_=bass.AP(xT, T * k0 * S + nt * CH,
                                [[S, T], [T * S, k1 - k0], [1, CH]]))

            def wq_load(eng, k0, k1):
                eng.dma_start(
                    out=wq_sb[:, k0:k1, :],
                    in_=bass.AP(wqkv, T * k0 * 3 * T,
                                [[3 * T, T], [T * 3 * T, k1 - k0],
                                 [1, 3 * T]]))

            wq_load(nc.sync, 0, KC)
            nc.sync.dma_start(out=id_sb[:], in_=iden[:])
            nc.sync.dma_start(out=id8_sb[:], in_=iden8[:])
            x_load(nc.sync, 1)
            nc.sync.dma_start(out=tri_sb[:], in_=tri[:])
            # scalar: wqb, x0 (chunk 0 feeds the very first matmuls), then
            # eT (needed by emit_R right after qkv chunks 0/1)
            nc.scalar.dma_start(out=wqb_sb[:], in_=wqkv_b[:])
            x_load(nc.scalar, 0)
            nc.scalar.dma_start(out=eT_sb[:], in_=eT[:])
            x_load(nc.scalar, 2)
            # gpsimd (slow software-DMA issue path): only late-needed loads
            x_load(nc.gpsimd, 3)
            nc.gpsimd.dma_start(out=wp_sb[:], in_=wproj[:])

            # preload the Exp table while the big DMAs run
            nc.vector.memset(warm_sb[:, 0:1], 0.0)
            nc.scalar.activation(warm_sb[:, 1:2], warm_sb[:, 0:1], AF.Exp)

            # ones columns for the fused rowsum
            nc.vector.memset(v_sb[:, :, 64:65], 1.0)
            nc.vector.memset(v_sb[:, :, 129:130], 1.0)
            nc.vector.memset(ones_sb[:], 1.0)

            with (
                tc.tile_pool(name="psS", bufs=2, space="PSUM") as psS,
                tc.tile_pool(name="psC", bufs=1, space="PSUM") as psC,
                tc.tile_pool(name="psA", bufs=2, space="PSUM") as psA,
                tc.tile_pool(name="sbR", bufs=4) as sbR,
                tc.tile_pool(name="sbB", bufs=2) as sbB,
                tc.tile_pool(name="sbP", bufs=8) as sbP,
                tc.tile_pool(name="sbN", bufs=2) as sbN,
                tc.tile_pool(name="sbO", bufs=4) as sbO,
            ):
                def qkv_chunk(nt):
                    for m in range(3):
                        acc = psA.tile([T, CH], F32, tag="aux",
                                       name=f"qkv{nt}_{m}")
                        for kc in range(KC):
                            nc.tensor.matmul(
                                acc[:],
                                wq_sb[:, kc, m * T:(m + 1) * T],
                                xc_sb[nt][:, kc, :],
                                start=(kc == 0), stop=(kc == KC - 1))
                        nc.vector.tensor_scalar_add(
                            qkvT_sb[:, m, nt * CH:(nt + 1) * CH], acc[:],
                            wqb_sb[:, m:m + 1])

                def vtr_chunk(nt):
                    # transpose this chunk's 4 v tiles into [j, d] layout
                    pv = psA.tile([T, CH], F32, tag="aux", name=f"vtr{nt}")
                    for jl in range(4):
                        jt = 4 * nt + jl
                        nc.tensor.matmul(
                            pv[:, jl * T:(jl + 1) * T],
                            qkvT_sb[:, 2, jt * T:(jt + 1) * T], id_sb[:],
                            start=True, stop=True, skip_group_check=True)
                    for jl in range(4):
                        jt = 4 * nt + jl
                        nc.vector.tensor_copy(v_sb[:, jt, 0:64],
                                              pv[:, jl * T:jl * T + 64])
                        nc.scalar.activation(v_sb[:, jt, 65:129],
                                             pv[:, jl * T + 64:jl * T + 128],
                                             AF.Copy)

                strip_tiles = {}

                def emit_R(qc, rs=(0, 1, 2, 3)):
                    if qc not in strip_tiles:
                        jwu = T * (4 * qc + 4)
                        strip_tiles[qc] = [
                            sbB.tile([T, 4, jwu], FP8, tag=f"strip{h}",
                                     name=f"stb{qc}_{h}") for h in range(2)]
                    jwu = T * (4 * qc + 4)
                    wq_ = [nc.sync, nc.gpsimd]        # skew DMA queue per head
                    for r in rs:
                        t = 4 * qc + r
                        m0, mhi = _m0(t), _mhi(t)
                        W = mhi - m0
                        jwt = S - m0          # causal band width
                        eRp = sbR.tile([T, 2, EW], FP8, tag="rawR",
                                       name=f"eRp{qc}_{r}")
                        for ck in range(math.ceil(jwt / CH)):
                            c0 = m0 + ck * CH
                            csz = min(CH, S - c0)
                            rp = [psA.tile([T, CH], F32, tag="aux",
                                           name=f"R{qc}_{r}_{ck}_{h}")
                                  for h in range(2)]
                            for h in range(2):
                                hp = slice(64 * h, 64 * h + 64)
                                nc.tensor.matmul(
                                    rp[h][:, 0:csz],
                                    qkvT_sb[hp, 0, t * T:(t + 1) * T],
                                    eT_sb[hp, c0:c0 + csz],
                                    start=True, stop=True,
                                    skip_group_check=True)
                            for h in range(2):
                                nc.vector.tensor_copy(
                                    eRp[:, h, ck * CH:ck * CH + csz],
                                    rp[h][:, 0:csz])
                        for h in range(2):
                            # non-causal tail -> exp ~ 0: the causal mask
                            # rides the skew for free (tri only for t=15,
                            # whose band has no room for a mask tail)
                            [nc.gpsimd, nc.vector][h].memset(
                                eRp[:, h, jwt:W], -200.0)
                            off_w = (PAD + CH * qc + T * r * (S + 1)
                                     + m0 - (S - 1))
                            wq_[h].dma_start(
                                out=bass.AP(bsk[h][qc], off_w,
                                            [[S + 1, T], [1, W]]),
                                in_=eRp[:, h, 0:W])
                            # bias strip r: same queue as its skew write, so
                            # it lands as soon as the write completes (and a
                            # chunk ahead of its consumers)
                            wq_[h].dma_start(
                                out=strip_tiles[qc][h][:, r, 0:jwu],
                                in_=bass.AP(bsk[h][qc], PAD + r * T * S,
                                            [[S, T], [1, jwu]]))

                def attn_chunk(qc, carry=(), next_qc=None, last=False):
                    # `carry` holds the previous chunk's deferred normalize +
                    # c_proj emission stages: the PE queue is strictly
                    # in-order, so emitting them at the boundary would stall
                    # the PE on the (long-latency) normalize chain while this
                    # chunk's ready scores sit behind it in the queue.
                    # Instead they interleave into the first few iterations.
                    strips = strip_tiles.pop(qc)
                    if carry:
                        carry[0]()
                    cx0 = psC.tile([T, CH], F32, tag="ctx0", name=f"cx0_{qc}")
                    cx1 = psC.tile([T, CH], F32, tag="ctx1", name=f"cx1_{qc}")
                    ctx_ps = [cx0, cx1]
                    jt_max = 4 * qc + 3

                    def emit_ctx(jt, il0, ext, pTp):
                        for h in range(2):
                            cx = ctx_ps[h]
                            nc.tensor.matmul(
                                cx[0:65, il0:il0 + ext],
                                v_sb[:, jt, 65 * h:65 * h + 65],
                                pTp[:, h, 0:ext],
                                start=(jt == 0), stop=(jt == jt_max),
                                skip_group_check=True)

                    pend = None
                    for jt in range(jt_max + 1):
                        if next_qc is not None and jt < 4:
                            # next chunk's R tiles, front-loaded so the skew
                            # DMA roundtrip completes well before that chunk
                            emit_R(next_qc, (jt,))
                        if 1 <= jt <= len(carry) - 1:
                            carry[jt]()
                        i0 = max(CH * qc, T * jt)
                        ext = CH * (qc + 1) - i0
                        il0 = i0 - CH * qc
                        diag = (i0 == T * jt)
                        r_lo = max(jt - 4 * qc, 0)
                        spp = psS.tile([T, 2 * CH], F32, tag="sT2",
                                       name=f"spp{qc}_{jt}")
                        for h in range(2):
                            hp = slice(64 * h, 64 * h + 64)
                            nc.tensor.matmul(
                                spp[:, h * CH:h * CH + ext],
                                qkvT_sb[hp, 1, jt * T:(jt + 1) * T],
                                qkvT_sb[hp, 0, i0:i0 + ext],
                                start=True, stop=False,
                                skip_group_check=True)
                        for h in range(2):
                            # bias add: strip[:, jt-tile].T via identity rhs,
                            # one 128-col piece per q tile covered by sp
                            for r in range(r_lo, 4):
                                co = h * CH + r * T - il0
                                nc.tensor.matmul(
                                    spp[:, co:co + T],
                                    strips[h][:, r, jt * T:(jt + 1) * T],
                                    id8_sb[:],
                                    start=False, stop=(r == 3),
                                    skip_group_check=True)
                        # ctx for the previous j tile: its exp completes
                        # while this tile's matmuls stream, so the PE never
                        # waits on the ACT engine
                        if pend is not None:
                            emit_ctx(*pend)
                        pTp = sbP.tile([T, 2, CH], BF16, tag="pT",
                                       name=f"pT{qc}_{jt}")
                        nc.scalar.activation(
                            pTp[:, :, 0:ext],
                            spp[:].rearrange("p (h c) -> p h c", h=2)
                            [:, :, 0:ext],
                            AF.Exp, scale=0.125)
                        if diag and jt == NJT - 1:
                            nc.vector.tensor_mul(pTp[:, 0, 0:T],
                                                 pTp[:, 0, 0:T], tri_sb[:])
                            nc.gpsimd.tensor_mul(pTp[:, 1, 0:T],
                                                 pTp[:, 1, 0:T], tri_sb[:])
                        if debug and qc == 0 and jt == 0:
                            nc.sync.dma_start(
                                out=bass.AP(d_pt, 0, [[2 * CH, T], [1, 2 * CH]]),
                                in_=pTp[:].rearrange("p a b -> p (a b)"))
                        pend = (jt, il0, ext, pTp)
                    emit_ctx(*pend)

                    # --- deferred normalize + merge heads + c_proj ---
                    def fin0():
                        # evacuate ctx PSUM (raw, incl. the rowsum row) right
                        # at the boundary so the next chunk's ctx
                        # accumulation gets its PSUM banks back immediately
                        for h in range(2):
                            nc.vector.tensor_copy(cxs[h][0:65, :],
                                                  ctx_ps[h][0:65, :])

                    def fin1(c0=0, c1=CH, shift_q=nc.sync):
                        # broadcast the RAW rowsum row across 64 partitions
                        # on PE (ones[1,64].T @ rowsum[1,512]), then the
                        # approx reciprocal runs 64-lane-parallel on PSUM
                        cw = c1 - c0
                        rsc = sbN.tile([T, 2, CH], BF16, tag="rsc",
                                       name=f"rsc{qc}_{c0}")
                        for h in range(2):
                            nc.vector.tensor_copy(rsc[64:65, h, c0:c1],
                                                  cxs[h][64:65, c0:c1])
                        bcp = [psA.tile([T, CH], F32, tag="aux",
                                        name=f"bcp{qc}_{h}_{c0}")
                               for h in range(2)]
                        bc = sbN.tile([T, 2, CH], F32, tag="bc",
                                      name=f"bc{qc}_{c0}")
                        for h in range(2):
                            nc.tensor.matmul(bcp[h][0:64, 0:cw],
                                             ones_sb[64:65, 0:64],
                                             rsc[64:65, h, c0:c1],
                                             start=True, stop=True)
                            nc.vector.reciprocal_approx_fast(
                                bc[0:64, h, c0:c1], bcp[h][0:64, 0:cw])
                        if debug and qc == 0 and c0 == 0:
                            for h in range(2):
                                nc.sync.dma_start(
                                    out=bass.AP(d_cxs, h * CH,
                                                [[2 * CH, T], [1, CH]]),
                                    in_=cxs[h][:])
                            nc.sync.dma_start(
                                out=bass.AP(d_rb, 0,
                                            [[2 * CH, T], [1, 2 * CH]]),
                                in_=bc[:].rearrange("p a b -> p (a b)"))
                        nc.vector.tensor_mul(
                            ctxn_sb[0:64, qc * CH + c0:qc * CH + c1],
                            cxs[0][0:64, c0:c1], bc[0:64, 0, c0:c1])
                        tmp1 = sbN.tile([T, CH], BF16, tag="tmp1",
                                        name=f"tmp1_{qc}_{c0}")
                        nc.vector.tensor_mul(tmp1[0:64, c0:c1],
                                             cxs[1][0:64, c0:c1],
                                             bc[0:64, 1, c0:c1])
                        # head 1 lives on partitions 64-127 of ctxn for the
                        # merged-head c_proj; engines cannot cross partitions
                        # but a SBUF->SBUF DMA can
                        shift_q.dma_start(
                            out=ctxn_sb[64:128, qc * CH + c0:qc * CH + c1],
                            in_=tmp1[0:64, c0:c1])

                    def cproj(rs_):
                        for r in rs_:
                            q0 = qc * CH + r * T
                            og = sbO.tile([T, HID], BF16, tag="og",
                                          name=f"og{qc}_{r}")
                            for oc in range(2):
                                pp = psA.tile([T, CH], F32, tag="aux",
                                              name=f"pp{qc}_{r}_{oc}")
                                nc.tensor.matmul(
                                    pp[:], ctxn_sb[:, q0:q0 + T],
                                    wp_sb[:, oc * CH:(oc + 1) * CH],
                                    start=True, stop=True)
                                if oc == 0:
                                    nc.vector.tensor_copy(
                                        og[:, oc * CH:(oc + 1) * CH], pp[:])
                                else:
                                    nc.scalar.activation(
                                        og[:, oc * CH:(oc + 1) * CH], pp[:],
                                        AF.Copy)
                            # last-processed chunk goes via the hardware DMA
                            # queues (sync/scalar) - the gpsimd software-DMA
                            # drain at NEFF teardown is slow
                            q_ = [nc.sync, nc.scalar] if qc == 0 \
                                else [nc.sync, nc.gpsimd]
                            q_[r % len(q_)].dma_start(
                                out=bass.AP(out, q0 * HID,
                                            [[HID, T], [1, HID]]),
                                in_=og[:])

                    cxs = [sbN.tile([T, CH], F32, tag=f"cxs{h}",
                                    name=f"cxs{qc}_{h}") for h in range(2)]
                    if last:
                        # final chunk: nothing left to interleave with, so
                        # pipeline the normalize by halves to unblock c_proj
                        # (and its out DMAs) as early as possible
                        return [fin0,
                                lambda: (fin1(0, CH // 2, nc.sync),
                                        cproj((0, 1))),
                                lambda: (fin1(CH // 2, CH, nc.scalar),
                                        cproj((2, 3)))]
                    return [fin0, fin1,
                            lambda: cproj((0, 1)), lambda: cproj((2, 3))]

                # ---- the fused pipeline ----
                # chunk order 1,2,3,0: any order is legal (chunks are
                # independent given qkv + their strips), and ending on the
                # smallest chunk (qc=0, 4 j-tiles) minimizes the tail
                qkv_chunk(0)
                qkv_chunk(1)
                emit_R(1)
                vtr_chunk(0)
                vtr_chunk(1)
                qkv_chunk(2)
                fin = attn_chunk(1, next_qc=2)
                qkv_chunk(3)
                vtr_chunk(2)
                fin = attn_chunk(2, fin, next_qc=3)
                vtr_chunk(3)
                fin = attn_chunk(3, fin, next_qc=0)
                fin = attn_chunk(0, fin, last=True)
                for f in fin:
                    f()

                if debug:
                    nc.sync.dma_start(
                        out=d_qkv[:],
                        in_=qkvT_sb[:].rearrange("p a b -> p (a b)"))
                    nc.sync.dma_start(
                        out=d_v[:], in_=v_sb[:].rearrange("p a b -> p (a b)"))
                    nc.sync.dma_start(out=d_ctxn[:], in_=ctxn_sb[:])

    nc.finalize()
    return nc


_NC_CACHE = {}


def _get_nc():
    if "nc" not in _NC_CACHE:
        _NC_CACHE["nc"] = build()
    return _NC_CACHE["nc"]


def _prep_core_inputs(x, c_attn_w, c_attn_b, c_proj_w, E):
    bf = ml_dtypes.bfloat16
    xT = np.ascontiguousarray(np.asarray(x)[0].T).astype(bf)     # [1024, 2048]
    c_attn_w = np.asarray(c_attn_w)
    c_attn_b = np.asarray(c_attn_b)
    c_proj_w = np.asarray(c_proj_w)
    E = np.asarray(E)
    # tri[j, q] = 1 if j <= q else 0 (upper triangular incl diagonal)
    tri = np.triu(np.ones((T, T), np.float32)).astype(bf)
    iden = np.eye(T, dtype=np.float32).astype(bf)
    iden8 = np.eye(T, dtype=np.float32).astype(ml_dtypes.float8_e4m3)
    maps = []
    for c in range(NCORES):
        qs = slice(T * c, T * (c + 1))
        wq = np.concatenate([
            c_attn_w[:, qs],
            c_attn_w[:, HID + T * c:HID + T * (c + 1)],
            c_attn_w[:, 2 * HID + T * c:2 * HID + T * (c + 1)],
        ], axis=1).astype(bf)                                    # [1024, 384]
        wqb = np.stack([
            c_attn_b[0, qs],
            c_attn_b[0, HID + T * c:HID + T * (c + 1)],
            c_attn_b[0, 2 * HID + T * c:2 * HID + T * (c + 1)],
        ], axis=1).astype(np.float32)                            # [128, 3]
        eTc = np.zeros((T, EW), np.float32)
        eTc[0:64, 0:S] = E[2 * c].T
        eTc[64:128, 0:S] = E[2 * c + 1].T
        wp = c_proj_w[T * c:T * (c + 1), :].astype(bf)           # [128, 1024]
        maps.append({
            "xT": xT, "wqkv": wq, "wqkv_b": wqb, "eT": eTc.astype(bf),
            "wproj": wp, "tri": tri, "iden": iden, "iden8": iden8,
        })
    return maps


def run_cores(inputs, trace=False, trace_kwargs=None):
    nc = _get_nc()
    maps = _prep_core_inputs(inputs["x"], inputs["c_attn_w"],
                             inputs["c_attn_b"], inputs["c_proj_w"],
                             inputs["E"])
    kw = {}
    if trace:
        kw["trace"] = True
        if trace_kwargs:
            kw.update(trace_kwargs)
    return run_bass_kernel_spmd(nc, maps, core_ids=list(range(NCORES)), **kw)


def kernel(**inputs):
    res = run_cores(inputs, trace=False)
    acc = np.zeros((S, HID), np.float32)
    for c in range(NCORES):
        acc += np.asarray(res.results[c]["out"]).astype(np.float32)
    acc += np.asarray(inputs["c_proj_b"]).astype(np.float32)
    return acc.reshape(1, S, HID)
